# revision 27
# baseline (speedup 1.0000x reference)
"""Trainium2 Bass kernel for nn_Eq2to2 (Maron et al. equivariant 2->2 layer).

Math (per batch n, with x[d,i,j] = inputs[n,i,j,d], W_b = coefs[:,:,b]):
  out[n,i,j,s] = LeakyReLU( sum_d W9[d,s] x[d,i,j] + W10[d,s] x[d,j,i]
                 + U[j,s] + V[i,s] + G[s] + bias[s]
                 + [i==j] (Dd[i,s] + E[s] + diag_bias[s]) )
  U = c@W5 + r@W6 + diag@W12, V = c@W7 + r@W8 + diag@W11
  Dd = diag@W0 + r@W2 + c@W3, G = tr@W13 + S@W14, E = tr@W1 + S@W4
  (r/c/diag = row sums / col sums / diagonal as [m, d]; tr/S totals)

Sharding (quadrant scheme): 8 cores = batch (4) x quadrant-pair (2).
Each core owns a transpose-CLOSED set of output positions, so both the
W9 path (x[i,j]) and the W10 path (x[j,i]) only touch that core's two
quadrants of x:
  q=0: diagonal quadrants  A = [0:64)x[0:64),   D = [64:128)x[64:128)
  q=1: off-diagonal        B = [0:64)x[64:128), C = [64:128)x[0:64)
All reduced terms (U/V/G/E/Dd) are computed on the host in fp32 (tiny)
and shipped as bf16 [S,128] vectors; the device only runs the two dense
mains + broadcast adds + LeakyReLU.

Device program per core (uniform across cores):
  xin [128, 16 banks x (straight 512 | transposed 512)] fp8e4m3 (2 MiB)
  per bank (8 rows x 64 cols of an output quadrant):
    PE:  psum  = W9 @ straight + W10 @ transposed + ident @ (U bcast)
    DVE: tmp   = psum + (V+G+bias bcast)          (bf16)
    Pool: tmp[diag positions] += (Dd+E+dbias)     (zeros on q=1 cores)
    ACT: out   = LeakyReLU(tmp)                   (bf16)
    DMA out [S, 512]
Host un-permutes the [S, 8192] per-core outputs into [B, M, M, S].
"""

import sys

if "/opt/trn_rl_repo" not in sys.path:
    sys.path.insert(0, "/opt/trn_rl_repo")

import numpy as np
import ml_dtypes

import concourse.bass as bass
import concourse.tile as tile
from concourse.tile import add_dep_helper
from concourse import bacc, mybir
from concourse.bass_utils import run_bass_kernel_spmd

B, M, D, S = 4, 128, 128, 128
H = 64               # quadrant side
NBANK = 16           # psum banks of 512 outputs (8 rows x 64 cols)
NCORES = 8
F32 = mybir.dt.float32
BF16 = mybir.dt.float16
F8 = mybir.dt.float8e4
AF = mybir.ActivationFunctionType
NEG_SLOPE = 0.01
BF_NP = np.float16
F8_NP = ml_dtypes.float8_e4m3


def _build_kernel():
    nc = bacc.Bacc(
        "TRN2", target_bir_lowering=False, debug=False, num_devices=NCORES
    )
    xind = nc.dram_tensor("xin", [D, NBANK * 1024], F8, kind="ExternalInput")
    # packed small tensors: [ident | u | vb | dc | wdr(fp8, bitcast)]
    smd = nc.dram_tensor("smalls", [M, 5 * M], BF16, kind="ExternalInput")
    out_t = nc.dram_tensor("out", [S, NBANK, 512], BF16, kind="ExternalOutput")

    with tile.TileContext(nc) as tc:
        _kernel_body(tc, nc, xind, smd, out_t)

    nc.compile()
    return nc


def _kernel_body(tc, nc, xind, smd, out_t):
    with (
        tc.tile_pool(name="const", bufs=1) as constp,
        tc.tile_pool(name="xt", bufs=1) as xtp,
        tc.tile_pool(name="psum", bufs=7, space="PSUM") as ppool,
        tc.tile_pool(name="warm", bufs=1, space="PSUM") as warmp,
        tc.tile_pool(name="tmp", bufs=4) as tmppool,
        tc.tile_pool(name="uvp", bufs=4) as uvpool,
        tc.tile_pool(name="osb", bufs=2) as opool,
    ):
        smalls = constp.tile([M, 5 * M], BF16)
        ident = smalls[:, 0:M]
        u_sb = smalls[:, M:2 * M]
        vb_sb = smalls[:, 2 * M:3 * M]
        dc_sb = smalls[:, 3 * M:4 * M]
        wdr = smalls[:, 4 * M:5 * M].bitcast(F8)   # [D, 2*S] fp8
        xin = xtp.tile([D, NBANK * 1024], F8)

        # smalls first on the SYNC ring so they beat the x chunks (FIFO
        # per ring), then the x chunks in bank order on the same ring.
        # Small leading chunks let the first matmuls start early.
        nc.sync.dma_start(smalls[:], smd.ap())
        CHUNKS = [1, 1, 2, 4, 4, 4]        # banks per chunk
        b0 = 0
        for nb in CHUNKS:
            nc.sync.dma_start(
                xin[:, b0 * 1024:(b0 + nb) * 1024],
                xind.ap()[:, b0 * 1024:(b0 + nb) * 1024],
            )
            b0 += nb

        # PE clock warmup + early Lrelu table load on memset scratch
        # (no DMA dependency, so these schedule immediately; keep the
        # lrelu-touch region disjoint from the warmup reads).  Dense
        # N=512 matmuls with a single stationary keep the PE array at
        # ~100% duty so the HAM clock-gate opens before real work.
        wsc = constp.tile([M, 512 + 16], BF16)
        nc.gpsimd.memset(wsc[:], 0.0)
        nc.scalar.activation(wsc[:, 512:512 + 16], wsc[:, 0:16], AF.Lrelu,
                             alpha=NEG_SLOPE)
        pw = warmp.tile([M, 512], F32)
        pe_prev = None

        def pe_chain(inst):
            # order-only dependency so the scheduler can't interleave
            # stationary-weight groups (avoids per-matmul LDWEIGHTS)
            nonlocal pe_prev
            if pe_prev is not None:
                add_dep_helper(inst.ins, pe_prev.ins, sync=False,
                               reason="pe order")
            pe_prev = inst

        for i in range(16):
            pe_chain(nc.tensor.matmul(
                pw[:], wsc[:, 0:M], wsc[:, 0:512],
                start=(i == 0), stop=(i == 15),
            ))

        x4 = xin[:].rearrange("d (b i c) -> d b i c", b=NBANK, i=2)
        wdr3 = wdr.rearrange("d (i s) -> d i s", i=2)
        DR = mybir.MatmulPerfMode.DoubleRow

        # bank categories:
        #   A (3..11): Pool pre-builds the combined U+V tile (Pool is
        #     otherwise idle); PE only does main+diag; DVE adds uv.
        #   B (0..2): PE adds U; DVE adds V (Pool would not be ready yet).
        #   C (12..15): PE adds everything; ACT applies Lrelu straight
        #     from PSUM (shortest possible tail for the last banks).
        CAT_A = set(range(3, 12))
        CAT_C = set(range(NBANK - 4, NBANK))
        DVE_LRELU = {5, 7, 9}   # cat-A banks whose lrelu runs on DVE
        uvt = {}
        for b in sorted(CAT_A):
            h, k = b // 8, b % 8
            a0 = h * H + 8 * k
            u3 = u_sb[:, h * H:(h + 1) * H].unsqueeze(1).broadcast_to(
                [S, 8, H]
            )
            v3 = vb_sb[:, a0:a0 + 8].unsqueeze(2).broadcast_to([S, 8, H])
            t = uvpool.tile([S, 512], BF16, name="uv", tag="uv")
            nc.gpsimd.tensor_add(
                t[:].rearrange("s (r c) -> s r c", r=8), u3, v3
            )
            uvt[b] = t

        osb = {}
        b0 = 0
        for nb in CHUNKS:
            banks = range(b0, b0 + nb)
            b0 += nb
            ps = {}
            for b in banks:
                # both mains in one K=256 fp8 DoubleRow matmul
                p = ppool.tile([S, 512], F32)
                pe_chain(nc.tensor.matmul(
                    p[:], wdr3, x4[:, b], start=True, stop=False,
                    perf_mode=DR,
                ))
                ps[b] = p
            for b in banks:
                if b in CAT_A:
                    continue
                h = b // 8
                ubc = u_sb[:, h * H:(h + 1) * H].unsqueeze(1).broadcast_to(
                    [S, 8, H]
                )
                pe_chain(nc.tensor.matmul(
                    ps[b][:].rearrange("s (r c) -> s r c", r=8),
                    ident, ubc, start=False, stop=False,
                ))
            for b in banks:
                # diagonal correction via a tiny N=8 accumulate matmul
                # into strided psum columns r*64 + (8k + r)
                h, k = b // 8, b % 8
                a0 = h * H + 8 * k
                pe_chain(nc.tensor.matmul(
                    ps[b][:, 8 * k:8 * k + 7 * 65 + 1:65],
                    ident, dc_sb[:, a0:a0 + 8], start=False,
                    stop=(b not in CAT_C),
                ))
            for b in banks:
                h, k = b // 8, b % 8
                a0 = h * H + 8 * k
                if b in CAT_C:
                    # tail banks: PE also folds the row term (strided
                    # psum write), so the drain is a single lrelu pass
                    vbc = vb_sb[:, a0:a0 + 8].unsqueeze(1).broadcast_to(
                        [S, H, 8]
                    )
                    pst = ps[b][:].rearrange(
                        "s (r c) -> s r c", r=8).transpose([0, 2, 1])
                    pe_chain(nc.tensor.matmul(
                        pst, ident, vbc, start=False, stop=True,
                    ))
                if b % 4 == 0:
                    osb[b] = opool.tile([S, 4 * 512], BF16, name="osb",
                                        tag="osb")
                dst = osb[b - b % 4][:, (b % 4) * 512:(b % 4 + 1) * 512]
                if b in CAT_C:
                    nc.scalar.activation(dst, ps[b][:], AF.Lrelu,
                                         alpha=NEG_SLOPE)
                else:
                    if b in CAT_A:
                        addend = uvt[b][:].rearrange("s (r c) -> s r c", r=8)
                    else:
                        addend = vb_sb[:, a0:a0 + 8].unsqueeze(2).broadcast_to(
                            [S, 8, H]
                        )
                    tmp = tmppool.tile([S, 512], BF16)
                    nc.vector.tensor_add(
                        tmp[:].rearrange("s (r c) -> s r c", r=8),
                        ps[b][:].rearrange("s (r c) -> s r c", r=8),
                        addend,
                    )
                    if b in DVE_LRELU:
                        nc.vector.scalar_tensor_tensor(
                            dst, tmp[:], NEG_SLOPE, tmp[:],
                            op0=mybir.AluOpType.mult,
                            op1=mybir.AluOpType.max,
                        )
                    else:
                        nc.scalar.activation(dst, tmp[:], AF.Lrelu,
                                             alpha=NEG_SLOPE)
                if b % 4 == 3:
                    # 4 KB/partition output DMA per bank quad, dispatched
                    # from the (idle) sync sequencer
                    nc.sync.dma_start(
                        out_t.ap()[:, b - 3:b + 1, :],
                        osb[b - 3][:].rearrange("s (j c) -> s j c", j=4),
                    )


_CACHE = {}


def _get_nc():
    if "nc" not in _CACHE:
        _CACHE["nc"] = _build_kernel()
    return _CACHE["nc"]


def _index_sets(q, h):
    if q == 0:
        iset = jset = np.arange(h * H, (h + 1) * H)
    elif h == 0:
        iset, jset = np.arange(0, H), np.arange(H, M)
    else:
        iset, jset = np.arange(H, M), np.arange(0, H)
    return iset, jset


def make_in_maps(inputs, coefs, bias, diag_bias):
    eye = np.eye(M, dtype=np.float32)
    # [d, (i, s)]: W9 at i=0, W10 at i=1 (DoubleRow stationary)
    wdr_np = np.ascontiguousarray(
        np.stack([coefs[:, :, 9], coefs[:, :, 10]], axis=1).reshape(D, 2 * S)
    ).astype(F8_NP)
    W = [coefs[:, :, b] for b in range(15)]

    in_maps = []
    for core in range(NCORES):
        n, q = core // 2, core % 2
        xd = np.ascontiguousarray(inputs[n].transpose(2, 0, 1))  # [d, i, j]
        r_ = xd.sum(axis=2)                 # [d, i]
        c_ = xd.sum(axis=1)                 # [d, j]
        dg = np.einsum('dii->di', xd)       # [d, i]
        tr = dg.sum(axis=1)
        tot = r_.sum(axis=1)
        U = c_.T @ W[5] + r_.T @ W[6] + dg.T @ W[12]    # [j, s]
        V = c_.T @ W[7] + r_.T @ W[8] + dg.T @ W[11]    # [i, s]
        Dd = dg.T @ W[0] + r_.T @ W[2] + c_.T @ W[3]    # [i, s]
        G = tr @ W[13] + tot @ W[14]
        E = tr @ W[1] + tot @ W[4]
        vbf = V + G[None, :] + bias[None, :]
        dcf = Dd + E[None, :] + diag_bias[None, :]

        xdT = xd.transpose(0, 2, 1)
        xin = np.empty((D, NBANK, 2, 512), dtype=F8_NP)
        sm = np.zeros((M, 4 * M), dtype=np.float32)
        sm[:, 0:M] = eye
        for h in range(2):
            iset, jset = _index_sets(q, h)
            st = xd[np.ix_(np.arange(D), iset, jset)]    # [d, 64, 64]
            tp = xdT[np.ix_(np.arange(D), iset, jset)]   # x[d, j, i]
            xin[:, 8 * h:8 * h + 8, 0, :] = st.reshape(D, 8, 512).astype(F8_NP)
            xin[:, 8 * h:8 * h + 8, 1, :] = tp.reshape(D, 8, 512).astype(F8_NP)
            sm[:, M + h * H:M + (h + 1) * H] = U[jset, :].T
            sm[:, 2 * M + h * H:2 * M + (h + 1) * H] = vbf[iset, :].T
            if q == 0:
                sm[:, 3 * M + h * H:3 * M + (h + 1) * H] = dcf[iset, :].T

        smb = np.empty((M, 5 * M), dtype=BF_NP)
        smb[:, 0:4 * M] = sm.astype(BF_NP)
        smb[:, 4 * M:5 * M] = wdr_np.view(BF_NP)   # fp8 pair -> one fp16 slot
        in_maps.append({
            "xin": np.ascontiguousarray(xin.reshape(D, NBANK * 1024)),
            "smalls": np.ascontiguousarray(smb),
        })
    return in_maps


def kernel(inputs, coefs, bias, diag_bias):
    inputs = np.ascontiguousarray(np.asarray(inputs, dtype=np.float32))
    coefs = np.asarray(coefs, dtype=np.float32)
    bias = np.asarray(bias, dtype=np.float32).reshape(-1)
    diag_bias = np.asarray(diag_bias, dtype=np.float32).reshape(-1)

    nc = _get_nc()
    in_maps = make_in_maps(inputs, coefs, bias, diag_bias)
    # the runtime occasionally reports a transient device-unrecoverable
    # state left over from a previous process; a retry clears it
    last_exc = None
    for attempt in range(3):
        try:
            res = run_bass_kernel_spmd(
                nc, in_maps, core_ids=list(range(NCORES))
            )
            break
        except Exception as e:  # noqa: BLE001
            last_exc = e
            import time as _time
            _time.sleep(10 * (attempt + 1))
    else:
        raise last_exc

    out = np.empty((B, M, M, S), dtype=np.float32)
    for core in range(NCORES):
        n, q = core // 2, core % 2
        r = res.results[core]["out"].astype(np.float32)  # [S, 16, 512]
        r = r.reshape(S, 2, 8, 8, H)
        for h in range(2):
            iset, jset = _index_sets(q, h)
            blk = r[:, h].reshape(S, H, H)               # [s, a, c]
            out[n][np.ix_(iset, jset)] = blk.transpose(1, 2, 0)
    return out
